# revision 56
# baseline (speedup 1.0000x reference)
"""Trainium2 Bass kernel for DepthwiseSeparableConv (depthwise 3x3 shared-kernel
conv -> channels-last memory-reinterpret -> pointwise 1x1 conv -> ReLU -> sync
BatchNorm), data-parallel over batch across 8 NeuronCores.

Self-contained: hardcodes shapes from the problem spec; imports only the
system-installed `concourse` (Bass/Tile) stack.

Per core (8 of 64 batches, 2 groups of 4):
  1. Host pre-transposes x to spatial-major [NGRP, 28, 112, 4*128] bf16 and
     pre-builds the three banded depthwise matrices A_d [112,112] bf16 plus
     bf16 pw^T, so the device does no transposes and no A-build.
  2. Depthwise conv as banded matmuls: z_blk(i) = sum_d A_d^T @ xt(i+d),
     d in {-1,0,+1} (block skip at the h-borders, w-border masked inside A).
     Both groups' convs run back-to-back so the in-order engine queues
     pipeline across groups.
  3. z bounces through DRAM in batch-interleaved [HW, GB, CIN] bf16 (full-rate
     1KB-row writes, one DMA per 2-block tile); the reinterpret re-read loads
     y in even/odd channel-permuted partition order (4 strided DMAs/batch,
     pw rows host-permuted to match, so the contraction is unchanged).
  4. Pointwise out = pw @ y on PE (bf16, f32 accum) into 4-bank PSUM tiles;
     ReLU+per-channel sums on ScalarE from PSUM (one merged instr per 4
     chunks); squares+sumsq spread over DVE (scalar_tensor_tensor w/ accum)
     and Pool (tensor_tensor + DVE tensor_scalar accum). Pre-BN activations
     stay in SBUF as bf16.
  5. (sum, sumsq) AllGather across 8 cores (cheaper than AllReduce in the
     collective fabric) + one-instruction local reduce; exact biased-variance
     BN affine split over DVE/ACT/Pool feeding the output-DMA tail.
"""

import os
import numpy as np
from contextlib import ExitStack

import concourse.bass as bass
import concourse.bacc as bacc
import concourse.mybir as mybir
from concourse import tile
from concourse.bass_utils import run_bass_kernel_spmd

F32 = mybir.dt.float32
BF16 = mybir.dt.bfloat16

B, CIN, COUT, H, W = 64, 128, 256, 56, 56
HW = H * W              # 3136
BLK = 112               # conv block rows (2*W)
NBLK = HW // BLK        # 28
NCORES = 8
BPC = B // NCORES       # 8 batches per core
GB = 4                  # batches per conv group
NGRP = BPC // GB        # 2
SPAN = 7                # x-load span (blocks per DMA)
NSPAN = NBLK // SPAN    # 4
NCH = 448               # pointwise moving chunk (1 PSUM bank holds 512 f32)
EPS = 1e-5
NTOT = float(B * HW)    # BN count


def build_nc():
    nc = bacc.Bacc(num_devices=NCORES)

    x_in = nc.declare_dram_parameter("xg", [NGRP, NBLK, BLK, GB * CIN], BF16,
                                     isOutput=False)
    a3 = nc.declare_dram_parameter("a3", [3, BLK, BLK], BF16, isOutput=False)
    pwt = nc.declare_dram_parameter("pwT", [CIN, COUT], BF16, isOutput=False)
    gam = nc.declare_dram_parameter("gamma", [COUT], F32, isOutput=False)
    bet = nc.declare_dram_parameter("beta", [COUT], F32, isOutput=False)
    out = nc.declare_dram_parameter("out", [BPC, COUT, HW], F32, isOutput=True)

    with ExitStack() as ctx:
        tc = ctx.enter_context(tile.TileContext(nc))
        const = ctx.enter_context(tc.tile_pool(name="const", bufs=1))
        xtpool = ctx.enter_context(tc.tile_pool(name="xt", bufs=4))
        zpool = ctx.enter_context(tc.tile_pool(name="z", bufs=4))
        ypool = ctx.enter_context(tc.tile_pool(name="y", bufs=3))
        opool = ctx.enter_context(tc.tile_pool(name="o", bufs=3))
        respool = ctx.enter_context(tc.tile_pool(name="res", bufs=2 * BPC))
        sbpool = ctx.enter_context(tc.tile_pool(name="scrbig", bufs=2))
        dram = ctx.enter_context(tc.tile_pool(name="dram", bufs=1, space="DRAM"))

        # ---- x span prefetch for group 0 goes first on the SP queue ----
        xs = [[None] * NSPAN for _ in range(NGRP)]
        for j in range(NSPAN):
            xt_ = xtpool.tile([BLK, SPAN * GB * CIN], BF16, tag="x",
                              name=f"xs0_{j}")
            nc.sync.dma_start(
                xt_[:].rearrange("p (i c) -> p i c", i=SPAN),
                x_in[0, SPAN * j:SPAN * (j + 1)].rearrange("i p c -> p i c"))
            xs[0][j] = xt_

        # ---- constants (host-prebuilt; tiny DMAs) ----
        A = {}
        for d in (-1, 0, 1):
            t = const.tile([BLK, BLK], BF16, tag=f"A{d}", name=f"A{d}")
            nc.sync.dma_start(t[:], a3[d + 1])
            A[d] = t
        pw_sb = const.tile([128, COUT], BF16, tag="pw")
        nc.sync.dma_start(pw_sb[:], pwt[:, :])
        gb_sb = const.tile([128, 4], F32, tag="gb")
        nc.sync.dma_start(gb_sb[:, 0:2], gam.ap().rearrange("(j p) -> p j", p=128))
        nc.sync.dma_start(gb_sb[:, 2:4], bet.ap().rearrange("(j p) -> p j", p=128))

        # stats slots: per oc, one column per (batch, chunk-pair)
        NSL = 4
        sums = [const.tile([128, BPC * NSL], F32, tag=f"sum{oc}",
                           name=f"sums{oc}") for oc in range(2)]
        sqs = [const.tile([128, BPC * NSL], F32, tag=f"sq{oc}",
                          name=f"sqs{oc}") for oc in range(2)]
        for oc in range(2):
            nc.gpsimd.memset(sums[oc][:], 0.0)
            nc.gpsimd.memset(sqs[oc][:], 0.0)
        epst = const.tile([128, 1], F32, tag="eps")
        nc.gpsimd.memset(epst[:], EPS)

        # z scratch interleaved [HW, GB, CIN]: full-rate 1KB-row writes; the
        # y re-read pays the small-chunk penalty instead (off critical path)
        zscr = [dram.tile([HW, GB, CIN], BF16, tag=f"zg{g}", name=f"zscr{g}")
                for g in range(NGRP)]
        st_in = dram.tile([128, 4], F32, tag="stin")
        st_gat = dram.tile([NCORES, 128, 4], F32, tag="stgat")

        res_tiles = [[None] * 2 for _ in range(BPC)]

        def read_y(y_sb, g, b4):
            """Reinterpret-read y from interleaved zscr[g] into even/odd
            channel-permuted partition order (pw rows are host-permuted to
            match, so the matmul contraction is unchanged).

            y[2q, 128j+cc] = Z[49q+j, cc]; y[2q+1, ...] starts mid-row 49q+24.
            """
            zv = zscr[g].rearrange("(q s) b c -> q s b c", q=64)
            nc.sync.dma_start(
                y_sb[0:64, 0:24 * CIN].rearrange("p (j c) -> p j c", c=CIN),
                zv[:, 0:24, b4, :])
            nc.sync.dma_start(y_sb[0:64, 24 * CIN:24 * CIN + 64],
                              zv[:, 24, b4, 0:64])
            nc.sync.dma_start(y_sb[64:128, 0:64], zv[:, 24, b4, 64:128])
            nc.sync.dma_start(
                y_sb[64:128, 64:HW].rearrange("p (j c) -> p j c", c=CIN),
                zv[:, 25:49, b4, :])

        # ---- conv / pointwise ----
        # sequential PSUM pools: conv 3x[112,1024] (6 banks) closes before
        # pw 2x[128,2048] (8 banks) opens
        y_tiles = [None] * BPC
        POOL_SQ = {1, 3, 5, 7, 9, 11}
        ACT_SQ = {15}

        def conv_group(g):
            def xt_blk(i):
                j, li = divmod(i, SPAN)
                return xs[g][j][:, 512 * li:512 * (li + 1)]

            # 2-block PSUM tiles: one [112,1024] zcopy per pair, ACT/DVE split
            for i2 in range(NBLK // 2):
                ps = ps_c.tile([BLK, 2 * GB * CIN], F32, tag="cv")
                for k in range(2):
                    i = 2 * i2 + k
                    deltas = [d for d in (-1, 0, 1) if 0 <= i + d < NBLK]
                    for idx, d in enumerate(deltas):
                        nc.tensor.matmul(
                            ps[:, 512 * k:512 * (k + 1)], A[d][:, :],
                            xt_blk(i + d),
                            start=(idx == 0), stop=(idx == len(deltas) - 1))
                z_sb = zpool.tile([BLK, 2 * GB * CIN], BF16, tag="z")
                if i2 % 2 == 0:
                    nc.vector.tensor_copy(z_sb[:], ps[:])
                else:
                    nc.scalar.activation(z_sb[:], ps[:],
                                         mybir.ActivationFunctionType.Copy)
                nc.sync.dma_start(
                    zscr[g][2 * BLK * i2:2 * BLK * (i2 + 1), :, :]
                    .rearrange("(k r) b c -> r k (b c)", k=2),
                    z_sb[:].rearrange("r (k v) -> r k v", k=2))

        def pw_batch(g, b4):
                b = GB * g + b4
                y_sb = y_tiles[b]
                for oc in range(2):
                    res = respool.tile([128, HW], BF16, tag="res")
                    res_tiles[b][oc] = res
                    u = 2 * b + oc
                    # chunk groups (0..3) and (4..6): one 4-bank PSUM tile and
                    # ONE relu+accum instr per group
                    for jj, js in enumerate([(0, 1, 2, 3), (4, 5, 6)]):
                        w = NCH * len(js)
                        ps = ps_p.tile([128, 2048], F32, tag="pw")
                        for k, j in enumerate(js):
                            nc.tensor.matmul(
                                ps[:, 512 * k:512 * k + NCH],
                                pw_sb[:, 128 * oc:128 * (oc + 1)],
                                y_sb[:, NCH * j:NCH * (j + 1)],
                                start=True, stop=True)
                        slot = b * NSL + jj
                        j0 = js[0]
                        ps_in = (ps[:].rearrange("p (k c) -> p k c", c=512)
                                 [:, 0:len(js), 0:NCH])
                        rs = (res[:, NCH * j0:NCH * j0 + w]
                              .rearrange("p (k c) -> p k c", c=NCH))
                        nc.scalar.activation(
                            rs, ps_in,
                            mybir.ActivationFunctionType.Relu,
                            accum_out=sums[oc][:, slot:slot + 1])
                    # one whole-tile square per unit; sumsq into slot b*NSL
                    sqslot = sqs[oc][:, b * NSL:b * NSL + 1]
                    if u in ACT_SQ:
                        sca = sbpool.tile([128, HW], BF16, tag="sb",
                                          name=f"sa{u}")
                        nc.scalar.activation(
                            sca[:], res[:],
                            mybir.ActivationFunctionType.Square,
                            accum_out=sqslot)
                    elif u in POOL_SQ:
                        scr_big = sbpool.tile([128, HW], BF16, tag="sb",
                                              name=f"sb{u}")
                        nc.gpsimd.tensor_tensor(
                            scr_big[:], res[:], res[:], mybir.AluOpType.mult)
                        sc2 = sbpool.tile([128, HW], BF16, tag="sb",
                                          name=f"sb2_{u}")
                        nc.vector.tensor_scalar(
                            sc2[:], scr_big[:], 1.0, 0.0,
                            mybir.AluOpType.mult, mybir.AluOpType.add,
                            accum_out=sqslot)
                    else:
                        sc = sbpool.tile([128, HW], BF16, tag="sb",
                                         name=f"sc{u}")
                        nc.vector.scalar_tensor_tensor(
                            out=sc[:], in0=res[:], scalar=1.0, in1=res[:],
                            op0=mybir.AluOpType.mult,
                            op1=mybir.AluOpType.mult,
                            accum_out=sqslot)

        # driver: conv(g0) -> y-g0/x-g1 -> conv(g1) -> pw(g0) -> pw(g1)
        ps_conv_ctx = tc.tile_pool(name="ps_c", bufs=3, space="PSUM")
        ps_c = ps_conv_ctx.__enter__()
        conv_group(0)
        for b4 in range(GB):
            y_sb = ypool.tile([128, HW], BF16, tag="y", name=f"y0_{b4}")
            y_tiles[b4] = y_sb
        read_y(y_tiles[0], 0, 0)
        for j in range(NSPAN):
            xt_ = xtpool.tile([BLK, SPAN * GB * CIN], BF16, tag="x",
                              name=f"xs1_{j}")
            nc.sync.dma_start(
                xt_[:].rearrange("p (i c) -> p i c", i=SPAN),
                x_in[1, SPAN * j:SPAN * (j + 1)].rearrange("i p c -> p i c"))
            xs[1][j] = xt_
        for b4 in range(1, GB):
            read_y(y_tiles[b4], 0, b4)
        conv_group(1)
        ps_conv_ctx.__exit__(None, None, None)
        ps_pw_ctx = tc.tile_pool(name="ps_p", bufs=2, space="PSUM")
        ps_p = ps_pw_ctx.__enter__()
        for b4 in range(GB):
            pw_batch(0, b4)
        for b4 in range(GB):
            y_sb = ypool.tile([128, HW], BF16, tag="y", name=f"y1_{b4}")
            y_tiles[GB + b4] = y_sb
            read_y(y_sb, 1, b4)
            pw_batch(1, b4)
        ps_pw_ctx.__exit__(None, None, None)

        # preload the Sqrt act-function table set during the ACT drain so the
        # post-collective chain doesn't pay the table swap
        warm = const.tile([128, 1], F32, tag="warm")
        nc.scalar.activation(warm[:], epst[:],
                             mybir.ActivationFunctionType.Sqrt)

        # ---- stats: local reduce -> AllGather -> local sum -> affine params
        red = const.tile([128, 4], F32, tag="red")
        allr = const.tile([128, 4 * NCORES], F32, tag="allr")
        acc = const.tile([128, 4], F32, tag="acc")
        me = const.tile([128, 4], F32, tag="me")    # mean0 mean1 msq0 msq1
        me2 = const.tile([128, 2], F32, tag="me2")
        var = const.tile([128, 2], F32, tag="var")
        rstd = const.tile([128, 2], F32, tag="rstd")
        sc_b = const.tile([128, 4], F32, tag="scb")  # scale0 scale1 nbias0 nbias1
        for oc in range(2):
            nc.vector.tensor_reduce(red[:, oc:oc + 1], sums[oc][:],
                                    axis=mybir.AxisListType.X,
                                    op=mybir.AluOpType.add)
            nc.vector.tensor_reduce(red[:, 2 + oc:3 + oc], sqs[oc][:],
                                    axis=mybir.AxisListType.X,
                                    op=mybir.AluOpType.add)
        no_cc = bool(os.environ.get("BASS_NO_CC"))
        nc.sync.dma_start(st_in[:], red[:])
        if no_cc:
            for r in range(NCORES):
                nc.sync.dma_start(st_gat[r], st_in[:])
        else:
            nc.gpsimd.collective_compute(
                "AllGather", mybir.AluOpType.bypass,
                replica_groups=[list(range(NCORES))],
                ins=[st_in[:].opt()], outs=[st_gat[:].opt()])
        # r-major readback (16B contiguous per core), then X-reduce over a
        # c-major view so one instruction sums the 8 cores
        nc.sync.dma_start(
            allr[:].rearrange("p (r c) -> p r c", r=NCORES),
            st_gat[:].rearrange("r p c -> p r c"))
        nc.vector.tensor_reduce(
            acc[:], allr[:].rearrange("p (r c) -> p c r", r=NCORES),
            axis=mybir.AxisListType.X, op=mybir.AluOpType.add)
        nc.vector.tensor_scalar(me[:], acc[:], 1.0 / NTOT, None,
                                mybir.AluOpType.mult)
        nc.vector.scalar_tensor_tensor(
            out=me2[:], in0=me[:, 0:2], scalar=1.0, in1=me[:, 0:2],
            op0=mybir.AluOpType.mult, op1=mybir.AluOpType.mult)
        nc.vector.tensor_tensor(var[:], me[:, 2:4], me2[:],
                                mybir.AluOpType.subtract)
        std = const.tile([128, 2], F32, tag="std")
        nc.scalar.activation(std[:], var[:],
                             mybir.ActivationFunctionType.Sqrt,
                             bias=epst[:])
        nc.vector.reciprocal(rstd[:], std[:])
        nc.vector.tensor_tensor(sc_b[:, 0:2], rstd[:], gb_sb[:, 0:2],
                                mybir.AluOpType.mult)
        nc.vector.tensor_tensor(sc_b[:, 2:4], me[:, 0:2], sc_b[:, 0:2],
                                mybir.AluOpType.mult)
        nc.vector.tensor_tensor(sc_b[:, 2:4], gb_sb[:, 2:4], sc_b[:, 2:4],
                                mybir.AluOpType.subtract)

        # ---- phase 2: affine + writeout, DVE/ACT/Pool rotation feeds DMA ----
        stage = int(os.environ.get("BASS_STAGE", "3"))
        for b in range(BPC):
            for oc in range(2):
                o_sb = opool.tile([128, HW], F32, tag="o")
                sel = (2 * b + oc) % 2
                if stage == 2:
                    nc.vector.tensor_copy(o_sb[:], res_tiles[b][oc][:])
                elif sel == 0:
                    nc.vector.tensor_scalar(
                        o_sb[:], res_tiles[b][oc][:],
                        sc_b[:, oc:oc + 1], sc_b[:, 2 + oc:3 + oc],
                        mybir.AluOpType.mult, mybir.AluOpType.add)
                elif sel == 1:
                    nc.scalar.activation(
                        o_sb[:], res_tiles[b][oc][:],
                        mybir.ActivationFunctionType.Identity,
                        bias=sc_b[:, 2 + oc:3 + oc],
                        scale=sc_b[:, oc:oc + 1])
                else:
                    nc.gpsimd.tensor_scalar(
                        o_sb[:], res_tiles[b][oc][:],
                        sc_b[:, oc:oc + 1], sc_b[:, 2 + oc:3 + oc],
                        mybir.AluOpType.mult, mybir.AluOpType.add)
                nc.sync.dma_start(out[b, 128 * oc:128 * (oc + 1), :], o_sb[:])

    nc.finalize()
    return nc


def _build_a3(dw_w):
    """Banded depthwise matrices A_d [rin(p), rout(f)] for d in (-1, 0, 1).

    A_d[p, f] = w[dh,dw] iff p == f + (56*dh+dw) - 112*d and the w-border
    is valid (0 <= f%56 + dw < 56). h-borders are exact via block skipping
    at i=0 / i=27 (cross-row taps there fall outside [0,112) or the w-mask).
    """
    w9 = np.asarray(dw_w, dtype=np.float32).reshape(3, 3)
    a3 = np.zeros((3, BLK, BLK), dtype=np.float32)
    f = np.arange(BLK)
    for di, d in enumerate((-1, 0, 1)):
        for dh in (-1, 0, 1):
            for dw in (-1, 0, 1):
                p = f + (56 * dh + dw) - 112 * d
                valid = (p >= 0) & (p < BLK) & (f % 56 + dw >= 0) & (f % 56 + dw < 56)
                a3[di, p[valid], f[valid]] = w9[dh + 1, dw + 1]
    return a3


_NC_CACHE = []


def kernel(x, dw_w, pw_w, gamma, beta):
    import ml_dtypes
    bf16 = ml_dtypes.bfloat16

    # [64,128,56,56] f32 -> [8 cores, NGRP, 3136, GB, 128] bf16 spatial-major
    xg = (np.asarray(x, dtype=np.float32).astype(bf16)
          .reshape(NCORES, NGRP, GB, CIN, HW)
          .transpose(0, 1, 4, 2, 3))
    xg = np.ascontiguousarray(xg).reshape(NCORES, NGRP, NBLK, BLK, GB * CIN)

    a3 = np.ascontiguousarray(_build_a3(dw_w).astype(bf16))
    # pw rows permuted even-channels-first to match the y-read partition order
    perm = np.r_[0:CIN:2, 1:CIN:2]
    pwT = np.ascontiguousarray(
        np.asarray(pw_w, dtype=np.float32).T.astype(bf16)[perm])
    gamma = np.ascontiguousarray(np.asarray(gamma, dtype=np.float32))
    beta = np.ascontiguousarray(np.asarray(beta, dtype=np.float32))

    if not _NC_CACHE:
        _NC_CACHE.append(build_nc())
    nc = _NC_CACHE[0]

    in_maps = []
    for r in range(NCORES):
        in_maps.append({"xg": np.ascontiguousarray(xg[r]), "a3": a3,
                        "pwT": pwT, "gamma": gamma, "beta": beta})

    br = run_bass_kernel_spmd(nc, in_maps, list(range(NCORES)))
    outs = [br.results[r]["out"].reshape(BPC, COUT, H, W) for r in range(NCORES)]
    return np.concatenate(outs, axis=0)


# revision 65
# speedup vs baseline: 1.0024x; 1.0024x over previous
"""Trainium2 Bass kernel for DepthwiseSeparableConv (depthwise 3x3 shared-kernel
conv -> channels-last memory-reinterpret -> pointwise 1x1 conv -> ReLU -> sync
BatchNorm), data-parallel over batch across 8 NeuronCores.

Self-contained: hardcodes shapes from the problem spec; imports only the
system-installed `concourse` (Bass/Tile) stack.

Per core (8 of 64 batches, 2 groups of 4):
  1. Host pre-transposes x to spatial-major [NGRP, 28, 112, 4*128] bf16 and
     pre-builds the three banded depthwise matrices A_d [112,112] bf16 plus
     bf16 pw^T, so the device does no transposes and no A-build.
  2. Depthwise conv as banded matmuls: z_blk(i) = sum_d A_d^T @ xt(i+d),
     d in {-1,0,+1} (block skip at the h-borders, w-border masked inside A).
     Both groups' convs run back-to-back so the in-order engine queues
     pipeline across groups.
  3. z bounces through DRAM in batch-interleaved [HW, GB, CIN] bf16 (full-rate
     1KB-row writes, one DMA per 2-block tile); the reinterpret re-read loads
     y in even/odd channel-permuted partition order (4 strided DMAs/batch,
     pw rows host-permuted to match, so the contraction is unchanged).
  4. Pointwise out = pw @ y on PE (bf16, f32 accum) into 4-bank PSUM tiles;
     ReLU+per-channel sums on ScalarE from PSUM (one merged instr per 4
     chunks); squares+sumsq spread over DVE (scalar_tensor_tensor w/ accum)
     and Pool (tensor_tensor + DVE tensor_scalar accum). Pre-BN activations
     stay in SBUF as bf16.
  5. (sum, sumsq) AllGather across 8 cores (cheaper than AllReduce in the
     collective fabric) + one-instruction local reduce; exact biased-variance
     BN affine split over DVE/ACT/Pool feeding the output-DMA tail.
"""

import os
import numpy as np
from contextlib import ExitStack

import concourse.bass as bass
import concourse.bacc as bacc
import concourse.mybir as mybir
from concourse import tile
from concourse.bass_utils import run_bass_kernel_spmd

F32 = mybir.dt.float32
BF16 = mybir.dt.bfloat16

B, CIN, COUT, H, W = 64, 128, 256, 56, 56
HW = H * W              # 3136
BLK = 112               # conv block rows (2*W)
NBLK = HW // BLK        # 28
NCORES = 8
BPC = B // NCORES       # 8 batches per core
GB = 4                  # batches per conv group
NGRP = BPC // GB        # 2
SPAN = 7                # x-load span (blocks per DMA)
NSPAN = NBLK // SPAN    # 4
NCH = 448               # pointwise moving chunk (1 PSUM bank holds 512 f32)
EPS = 1e-5
NTOT = float(B * HW)    # BN count


def build_nc():
    nc = bacc.Bacc(num_devices=NCORES)

    x_in = nc.declare_dram_parameter("xg", [NGRP, NBLK, BLK, GB * CIN], BF16,
                                     isOutput=False)
    a3 = nc.declare_dram_parameter("a3", [3, BLK, BLK], BF16, isOutput=False)
    pwt = nc.declare_dram_parameter("pwT", [CIN, COUT], BF16, isOutput=False)
    gam = nc.declare_dram_parameter("gamma", [COUT], F32, isOutput=False)
    bet = nc.declare_dram_parameter("beta", [COUT], F32, isOutput=False)
    out = nc.declare_dram_parameter("out", [BPC, COUT, HW], F32, isOutput=True)

    with ExitStack() as ctx:
        tc = ctx.enter_context(tile.TileContext(nc))
        const = ctx.enter_context(tc.tile_pool(name="const", bufs=1))
        xtpool = ctx.enter_context(tc.tile_pool(name="xt", bufs=4))
        zpool = ctx.enter_context(tc.tile_pool(name="z", bufs=6))
        ypool = ctx.enter_context(tc.tile_pool(name="y", bufs=3))
        opool = ctx.enter_context(tc.tile_pool(name="o", bufs=3))
        respool = ctx.enter_context(tc.tile_pool(name="res", bufs=2 * BPC))
        sbpool = ctx.enter_context(tc.tile_pool(name="scrbig", bufs=2))
        dram = ctx.enter_context(tc.tile_pool(name="dram", bufs=1, space="DRAM"))

        # ---- x span prefetch for group 0 goes first on the SP queue ----
        xs = [[None] * NSPAN for _ in range(NGRP)]
        for j in range(NSPAN):
            xt_ = xtpool.tile([BLK, SPAN * GB * CIN], BF16, tag="x",
                              name=f"xs0_{j}")
            nc.sync.dma_start(
                xt_[:].rearrange("p (i c) -> p i c", i=SPAN),
                x_in[0, SPAN * j:SPAN * (j + 1)].rearrange("i p c -> p i c"))
            xs[0][j] = xt_

        # ---- constants (host-prebuilt; tiny DMAs) ----
        A = {}
        for d in (-1, 0, 1):
            t = const.tile([BLK, BLK], BF16, tag=f"A{d}", name=f"A{d}")
            nc.sync.dma_start(t[:], a3[d + 1])
            A[d] = t
        pw_sb = const.tile([128, COUT], BF16, tag="pw")
        nc.sync.dma_start(pw_sb[:], pwt[:, :])
        gb_sb = const.tile([128, 4], F32, tag="gb")
        nc.sync.dma_start(gb_sb[:, 0:2], gam.ap().rearrange("(j p) -> p j", p=128))
        nc.sync.dma_start(gb_sb[:, 2:4], bet.ap().rearrange("(j p) -> p j", p=128))

        # stats slots: per oc, one column per (batch, chunk-pair)
        NSL = 4
        sums = [const.tile([128, BPC * NSL], F32, tag=f"sum{oc}",
                           name=f"sums{oc}") for oc in range(2)]
        sqs = [const.tile([128, BPC * NSL], F32, tag=f"sq{oc}",
                          name=f"sqs{oc}") for oc in range(2)]
        for oc in range(2):
            nc.gpsimd.memset(sums[oc][:], 0.0)
            nc.gpsimd.memset(sqs[oc][:], 0.0)
        epst = const.tile([128, 1], F32, tag="eps")
        nc.gpsimd.memset(epst[:], EPS)

        # z scratch interleaved [HW, GB, CIN]: full-rate 1KB-row writes; the
        # y re-read pays the small-chunk penalty instead (off critical path)
        zscr = [dram.tile([HW, GB, CIN], BF16, tag=f"zg{g}", name=f"zscr{g}")
                for g in range(NGRP)]
        st_in = dram.tile([128, 4], F32, tag="stin")
        st_gat = dram.tile([NCORES, 128, 4], F32, tag="stgat")

        res_tiles = [[None] * 2 for _ in range(BPC)]

        def read_y(y_sb, g, b4):
            """Reinterpret-read y from interleaved zscr[g] into even/odd
            channel-permuted partition order (pw rows are host-permuted to
            match, so the matmul contraction is unchanged).

            y[2q, 128j+cc] = Z[49q+j, cc]; y[2q+1, ...] starts mid-row 49q+24.
            """
            zv = zscr[g].rearrange("(q s) b c -> q s b c", q=64)
            nc.sync.dma_start(
                y_sb[0:64, 0:24 * CIN].rearrange("p (j c) -> p j c", c=CIN),
                zv[:, 0:24, b4, :])
            nc.sync.dma_start(y_sb[0:64, 24 * CIN:24 * CIN + 64],
                              zv[:, 24, b4, 0:64])
            nc.sync.dma_start(y_sb[64:128, 0:64], zv[:, 24, b4, 64:128])
            nc.sync.dma_start(
                y_sb[64:128, 64:HW].rearrange("p (j c) -> p j c", c=CIN),
                zv[:, 25:49, b4, :])

        # ---- conv / pointwise ----
        # sequential PSUM pools: conv 3x[112,1024] (6 banks) closes before
        # pw 2x[128,2048] (8 banks) opens
        y_tiles = [None] * BPC
        POOL_SQ = {1, 3, 5, 7, 9, 11}
        ACT_SQ = {15}
        DVE_RELU = {0}

        def conv_group(g):
            def xt_blk(i):
                j, li = divmod(i, SPAN)
                return xs[g][j][:, 512 * li:512 * (li + 1)]

            # 2-block PSUM tiles: one [112,1024] zcopy per pair, ACT/DVE split
            for i2 in range(NBLK // 2):
                ps = ps_c.tile([BLK, 2 * GB * CIN], F32, tag="cv")
                for k in range(2):
                    i = 2 * i2 + k
                    deltas = [d for d in (-1, 0, 1) if 0 <= i + d < NBLK]
                    for idx, d in enumerate(deltas):
                        nc.tensor.matmul(
                            ps[:, 512 * k:512 * (k + 1)], A[d][:, :],
                            xt_blk(i + d),
                            start=(idx == 0), stop=(idx == len(deltas) - 1))
                z_sb = zpool.tile([BLK, 2 * GB * CIN], BF16, tag="z")
                if i2 % 2 == 0:
                    nc.vector.tensor_copy(z_sb[:], ps[:])
                else:
                    nc.scalar.activation(z_sb[:], ps[:],
                                         mybir.ActivationFunctionType.Copy)
                nc.sync.dma_start(
                    zscr[g][2 * BLK * i2:2 * BLK * (i2 + 1), :, :]
                    .rearrange("(k r) b c -> r k (b c)", k=2),
                    z_sb[:].rearrange("r (k v) -> r k v", k=2))

        def pw_batch(g, b4):
                b = GB * g + b4
                y_sb = y_tiles[b]
                for oc in range(2):
                    res = respool.tile([128, HW], BF16, tag="res")
                    res_tiles[b][oc] = res
                    u = 2 * b + oc
                    # chunk groups (0..3) and (4..6): one 4-bank PSUM tile and
                    # ONE relu+accum instr per group
                    for jj, js in enumerate([(0, 1, 2, 3), (4, 5, 6)]):
                        w = NCH * len(js)
                        ps = ps_p.tile([128, 2048], F32, tag="pw")
                        for k, j in enumerate(js):
                            nc.tensor.matmul(
                                ps[:, 512 * k:512 * k + NCH],
                                pw_sb[:, 128 * oc:128 * (oc + 1)],
                                y_sb[:, NCH * j:NCH * (j + 1)],
                                start=True, stop=True)
                        slot = b * NSL + jj
                        j0 = js[0]
                        ps_in = (ps[:].rearrange("p (k c) -> p k c", c=512)
                                 [:, 0:len(js), 0:NCH])
                        rs = (res[:, NCH * j0:NCH * j0 + w]
                              .rearrange("p (k c) -> p k c", c=NCH))
                        if u in DVE_RELU:
                            nc.vector.tensor_scalar(
                                rs, ps_in, 0.0, 0.0,
                                mybir.AluOpType.max, mybir.AluOpType.add,
                                accum_out=sums[oc][:, slot:slot + 1])
                        else:
                            nc.scalar.activation(
                                rs, ps_in,
                                mybir.ActivationFunctionType.Relu,
                                accum_out=sums[oc][:, slot:slot + 1])
                    # one whole-tile square per unit; sumsq into slot b*NSL
                    sqslot = sqs[oc][:, b * NSL:b * NSL + 1]
                    if u in ACT_SQ:
                        sca = sbpool.tile([128, HW], BF16, tag="sb",
                                          name=f"sa{u}")
                        nc.scalar.activation(
                            sca[:], res[:],
                            mybir.ActivationFunctionType.Square,
                            accum_out=sqslot)
                    elif u in POOL_SQ:
                        scr_big = sbpool.tile([128, HW], BF16, tag="sb",
                                              name=f"sb{u}")
                        nc.gpsimd.tensor_tensor(
                            scr_big[:], res[:], res[:], mybir.AluOpType.mult)
                        sc2 = sbpool.tile([128, HW], BF16, tag="sb",
                                          name=f"sb2_{u}")
                        nc.vector.tensor_scalar(
                            sc2[:], scr_big[:], 1.0, 0.0,
                            mybir.AluOpType.mult, mybir.AluOpType.add,
                            accum_out=sqslot)
                    else:
                        sc = sbpool.tile([128, HW], BF16, tag="sb",
                                         name=f"sc{u}")
                        nc.vector.scalar_tensor_tensor(
                            out=sc[:], in0=res[:], scalar=1.0, in1=res[:],
                            op0=mybir.AluOpType.mult,
                            op1=mybir.AluOpType.mult,
                            accum_out=sqslot)

        # driver: conv(g0) -> y-g0/x-g1 -> conv(g1) -> pw(g0) -> pw(g1)
        ps_conv_ctx = tc.tile_pool(name="ps_c", bufs=3, space="PSUM")
        ps_c = ps_conv_ctx.__enter__()
        conv_group(0)
        for b4 in range(GB):
            y_sb = ypool.tile([128, HW], BF16, tag="y", name=f"y0_{b4}")
            y_tiles[b4] = y_sb
        read_y(y_tiles[0], 0, 0)
        for j in range(NSPAN):
            xt_ = xtpool.tile([BLK, SPAN * GB * CIN], BF16, tag="x",
                              name=f"xs1_{j}")
            nc.sync.dma_start(
                xt_[:].rearrange("p (i c) -> p i c", i=SPAN),
                x_in[1, SPAN * j:SPAN * (j + 1)].rearrange("i p c -> p i c"))
            xs[1][j] = xt_
        for b4 in range(1, GB):
            read_y(y_tiles[b4], 0, b4)
        conv_group(1)
        ps_conv_ctx.__exit__(None, None, None)
        ps_pw_ctx = tc.tile_pool(name="ps_p", bufs=2, space="PSUM")
        ps_p = ps_pw_ctx.__enter__()
        for b4 in range(GB):
            pw_batch(0, b4)
        for b4 in range(GB):
            y_sb = ypool.tile([128, HW], BF16, tag="y", name=f"y1_{b4}")
            y_tiles[GB + b4] = y_sb
            read_y(y_sb, 1, b4)
            pw_batch(1, b4)
        ps_pw_ctx.__exit__(None, None, None)

        # preload the Sqrt act-function table set during the ACT drain so the
        # post-collective chain doesn't pay the table swap
        warm = const.tile([128, 1], F32, tag="warm")
        nc.scalar.activation(warm[:], epst[:],
                             mybir.ActivationFunctionType.Sqrt)

        # ---- stats: local reduce -> AllGather -> local sum -> affine params
        red = const.tile([128, 4], F32, tag="red")
        allr = const.tile([128, 4 * NCORES], F32, tag="allr")
        acc = const.tile([128, 4], F32, tag="acc")
        me = const.tile([128, 4], F32, tag="me")    # mean0 mean1 msq0 msq1
        me2 = const.tile([128, 2], F32, tag="me2")
        var = const.tile([128, 2], F32, tag="var")
        rstd = const.tile([128, 2], F32, tag="rstd")
        sc_b = const.tile([128, 4], F32, tag="scb")  # scale0 scale1 nbias0 nbias1
        for oc in range(2):
            nc.vector.tensor_reduce(red[:, oc:oc + 1], sums[oc][:],
                                    axis=mybir.AxisListType.X,
                                    op=mybir.AluOpType.add)
            nc.vector.tensor_reduce(red[:, 2 + oc:3 + oc], sqs[oc][:],
                                    axis=mybir.AxisListType.X,
                                    op=mybir.AluOpType.add)
        no_cc = bool(os.environ.get("BASS_NO_CC"))
        nc.sync.dma_start(st_in[:], red[:])
        if no_cc:
            for r in range(NCORES):
                nc.sync.dma_start(st_gat[r], st_in[:])
        else:
            nc.gpsimd.collective_compute(
                "AllGather", mybir.AluOpType.bypass,
                replica_groups=[list(range(NCORES))],
                ins=[st_in[:].opt()], outs=[st_gat[:].opt()])
        # r-major readback (16B contiguous per core), then X-reduce over a
        # c-major view so one instruction sums the 8 cores
        nc.sync.dma_start(
            allr[:].rearrange("p (r c) -> p r c", r=NCORES),
            st_gat[:].rearrange("r p c -> p r c"))
        nc.vector.tensor_reduce(
            acc[:], allr[:].rearrange("p (r c) -> p c r", r=NCORES),
            axis=mybir.AxisListType.X, op=mybir.AluOpType.add)
        nc.vector.tensor_scalar(me[:], acc[:], 1.0 / NTOT, None,
                                mybir.AluOpType.mult)
        nc.vector.scalar_tensor_tensor(
            out=me2[:], in0=me[:, 0:2], scalar=1.0, in1=me[:, 0:2],
            op0=mybir.AluOpType.mult, op1=mybir.AluOpType.mult)
        nc.vector.tensor_tensor(var[:], me[:, 2:4], me2[:],
                                mybir.AluOpType.subtract)
        std = const.tile([128, 2], F32, tag="std")
        nc.scalar.activation(std[:], var[:],
                             mybir.ActivationFunctionType.Sqrt,
                             bias=epst[:])
        nc.vector.reciprocal(rstd[:], std[:])
        nc.vector.tensor_tensor(sc_b[:, 0:2], rstd[:], gb_sb[:, 0:2],
                                mybir.AluOpType.mult)
        nc.vector.tensor_tensor(sc_b[:, 2:4], me[:, 0:2], sc_b[:, 0:2],
                                mybir.AluOpType.mult)
        nc.vector.tensor_tensor(sc_b[:, 2:4], gb_sb[:, 2:4], sc_b[:, 2:4],
                                mybir.AluOpType.subtract)

        # ---- phase 2: affine + writeout, DVE/ACT/Pool rotation feeds DMA ----
        stage = int(os.environ.get("BASS_STAGE", "3"))
        for b in range(BPC):
            for oc in range(2):
                o_sb = opool.tile([128, HW], F32, tag="o")
                sel = (2 * b + oc) % 2
                if stage == 2:
                    nc.vector.tensor_copy(o_sb[:], res_tiles[b][oc][:])
                elif sel == 0:
                    nc.vector.tensor_scalar(
                        o_sb[:], res_tiles[b][oc][:],
                        sc_b[:, oc:oc + 1], sc_b[:, 2 + oc:3 + oc],
                        mybir.AluOpType.mult, mybir.AluOpType.add)
                elif sel == 1:
                    nc.scalar.activation(
                        o_sb[:], res_tiles[b][oc][:],
                        mybir.ActivationFunctionType.Identity,
                        bias=sc_b[:, 2 + oc:3 + oc],
                        scale=sc_b[:, oc:oc + 1])
                else:
                    nc.gpsimd.tensor_scalar(
                        o_sb[:], res_tiles[b][oc][:],
                        sc_b[:, oc:oc + 1], sc_b[:, 2 + oc:3 + oc],
                        mybir.AluOpType.mult, mybir.AluOpType.add)
                nc.sync.dma_start(out[b, 128 * oc:128 * (oc + 1), :], o_sb[:])

    nc.finalize()
    return nc


def _build_a3(dw_w):
    """Banded depthwise matrices A_d [rin(p), rout(f)] for d in (-1, 0, 1).

    A_d[p, f] = w[dh,dw] iff p == f + (56*dh+dw) - 112*d and the w-border
    is valid (0 <= f%56 + dw < 56). h-borders are exact via block skipping
    at i=0 / i=27 (cross-row taps there fall outside [0,112) or the w-mask).
    """
    w9 = np.asarray(dw_w, dtype=np.float32).reshape(3, 3)
    a3 = np.zeros((3, BLK, BLK), dtype=np.float32)
    f = np.arange(BLK)
    for di, d in enumerate((-1, 0, 1)):
        for dh in (-1, 0, 1):
            for dw in (-1, 0, 1):
                p = f + (56 * dh + dw) - 112 * d
                valid = (p >= 0) & (p < BLK) & (f % 56 + dw >= 0) & (f % 56 + dw < 56)
                a3[di, p[valid], f[valid]] = w9[dh + 1, dw + 1]
    return a3


_NC_CACHE = []


def kernel(x, dw_w, pw_w, gamma, beta):
    import ml_dtypes
    bf16 = ml_dtypes.bfloat16

    # [64,128,56,56] f32 -> [8 cores, NGRP, 3136, GB, 128] bf16 spatial-major
    xg = (np.asarray(x, dtype=np.float32).astype(bf16)
          .reshape(NCORES, NGRP, GB, CIN, HW)
          .transpose(0, 1, 4, 2, 3))
    xg = np.ascontiguousarray(xg).reshape(NCORES, NGRP, NBLK, BLK, GB * CIN)

    a3 = np.ascontiguousarray(_build_a3(dw_w).astype(bf16))
    # pw rows permuted even-channels-first to match the y-read partition order
    perm = np.r_[0:CIN:2, 1:CIN:2]
    pwT = np.ascontiguousarray(
        np.asarray(pw_w, dtype=np.float32).T.astype(bf16)[perm])
    gamma = np.ascontiguousarray(np.asarray(gamma, dtype=np.float32))
    beta = np.ascontiguousarray(np.asarray(beta, dtype=np.float32))

    if not _NC_CACHE:
        _NC_CACHE.append(build_nc())
    nc = _NC_CACHE[0]

    in_maps = []
    for r in range(NCORES):
        in_maps.append({"xg": np.ascontiguousarray(xg[r]), "a3": a3,
                        "pwT": pwT, "gamma": gamma, "beta": beta})

    br = run_bass_kernel_spmd(nc, in_maps, list(range(NCORES)))
    outs = [br.results[r]["out"].reshape(BPC, COUT, H, W) for r in range(NCORES)]
    return np.concatenate(outs, axis=0)


# revision 75
# speedup vs baseline: 1.0232x; 1.0208x over previous
"""Trainium2 Bass kernel for DepthwiseSeparableConv (depthwise 3x3 shared-kernel
conv -> channels-last memory-reinterpret -> pointwise 1x1 conv -> ReLU -> sync
BatchNorm), data-parallel over batch across 8 NeuronCores.

Self-contained: hardcodes shapes from the problem spec; imports only the
system-installed `concourse` (Bass/Tile) stack.

Per core (8 of 64 batches, 2 groups of 4):
  1. Host pre-transposes x to spatial-major [NGRP, 28, 112, 4*128] bf16 and
     pre-builds the three banded depthwise matrices A_d [112,112] bf16 plus
     bf16 pw^T, so the device does no transposes and no A-build.
  2. Depthwise conv as banded matmuls: z_blk(i) = sum_d A_d^T @ xt(i+d),
     d in {-1,0,+1} (block skip at the h-borders, w-border masked inside A).
     Both groups' convs run back-to-back so the in-order engine queues
     pipeline across groups.
  3. z bounces through DRAM in batch-interleaved [HW, GB, CIN] bf16 (full-rate
     1KB-row writes, one DMA per 2-block tile); the reinterpret re-read loads
     y in even/odd channel-permuted partition order (4 strided DMAs/batch,
     pw rows host-permuted to match, so the contraction is unchanged).
  4. Pointwise out = pw @ y on PE (bf16, f32 accum) into 4-bank PSUM tiles;
     ReLU+per-channel sums on ScalarE from PSUM (one merged instr per 4
     chunks); squares+sumsq spread over DVE (scalar_tensor_tensor w/ accum)
     and Pool (tensor_tensor + DVE tensor_scalar accum). Pre-BN activations
     stay in SBUF as bf16.
  5. (sum, sumsq) AllGather across 8 cores (cheaper than AllReduce in the
     collective fabric) + one-instruction local reduce; exact biased-variance
     BN affine split over DVE/ACT/Pool feeding the output-DMA tail.
"""

import os
import numpy as np
from contextlib import ExitStack

import concourse.bass as bass
import concourse.bacc as bacc
import concourse.mybir as mybir
from concourse import tile
from concourse.bass_utils import run_bass_kernel_spmd

F32 = mybir.dt.float32
BF16 = mybir.dt.bfloat16

B, CIN, COUT, H, W = 64, 128, 256, 56, 56
HW = H * W              # 3136
BLK = 112               # conv block rows (2*W)
NBLK = HW // BLK        # 28
NCORES = 8
BPC = B // NCORES       # 8 batches per core
GB = 4                  # batches per conv group
NGRP = BPC // GB        # 2
SPAN = 7                # x-load span (blocks per DMA)
NSPAN = NBLK // SPAN    # 4
NCH = 448               # pointwise moving chunk (1 PSUM bank holds 512 f32)
EPS = 1e-5
NTOT = float(B * HW)    # BN count


def build_nc():
    nc = bacc.Bacc(num_devices=NCORES)

    x_in = nc.declare_dram_parameter("xg", [NGRP, NBLK, BLK, GB * CIN], BF16,
                                     isOutput=False)
    a3 = nc.declare_dram_parameter("a3", [3, BLK, BLK], BF16, isOutput=False)
    pwt = nc.declare_dram_parameter("pwT", [CIN, COUT], BF16, isOutput=False)
    gam = nc.declare_dram_parameter("gamma", [COUT], F32, isOutput=False)
    bet = nc.declare_dram_parameter("beta", [COUT], F32, isOutput=False)
    out = nc.declare_dram_parameter("out", [BPC, COUT, HW], F32, isOutput=True)

    with ExitStack() as ctx:
        tc = ctx.enter_context(tile.TileContext(nc))
        const = ctx.enter_context(tc.tile_pool(name="const", bufs=1))
        xtpool = ctx.enter_context(tc.tile_pool(name="xt", bufs=4))
        zpool = ctx.enter_context(tc.tile_pool(name="z", bufs=6))
        ypool = ctx.enter_context(tc.tile_pool(name="y", bufs=3))
        opool = ctx.enter_context(tc.tile_pool(name="o", bufs=3))
        respool = ctx.enter_context(tc.tile_pool(name="res", bufs=2 * BPC))
        sbpool = ctx.enter_context(tc.tile_pool(name="scrbig", bufs=2))
        dram = ctx.enter_context(tc.tile_pool(name="dram", bufs=1, space="DRAM"))

        # ---- x span prefetch for group 0 goes first on the SP queue ----
        xs = [[None] * NSPAN for _ in range(NGRP)]
        for j in range(NSPAN):
            xt_ = xtpool.tile([BLK, SPAN * GB * CIN], BF16, tag="x",
                              name=f"xs0_{j}")
            nc.sync.dma_start(
                xt_[:].rearrange("p (i c) -> p i c", i=SPAN),
                x_in[0, SPAN * j:SPAN * (j + 1)].rearrange("i p c -> p i c"))
            xs[0][j] = xt_

        # ---- constants (host-prebuilt; tiny DMAs) ----
        A = {}
        for d in (-1, 0, 1):
            t = const.tile([BLK, BLK], BF16, tag=f"A{d}", name=f"A{d}")
            nc.sync.dma_start(t[:], a3[d + 1])
            A[d] = t
        pw_sb = const.tile([128, COUT], BF16, tag="pw")
        nc.sync.dma_start(pw_sb[:], pwt[:, :])
        gb_sb = const.tile([128, 4], F32, tag="gb")
        nc.sync.dma_start(gb_sb[:, 0:2], gam.ap().rearrange("(j p) -> p j", p=128))
        nc.sync.dma_start(gb_sb[:, 2:4], bet.ap().rearrange("(j p) -> p j", p=128))

        # stats slots: per oc, one column per (batch, chunk-pair)
        NSL = 4
        sums = [const.tile([128, BPC * NSL], F32, tag=f"sum{oc}",
                           name=f"sums{oc}") for oc in range(2)]
        sqs = [const.tile([128, BPC * NSL], F32, tag=f"sq{oc}",
                          name=f"sqs{oc}") for oc in range(2)]
        for oc in range(2):
            nc.gpsimd.memset(sums[oc][:], 0.0)
            nc.gpsimd.memset(sqs[oc][:], 0.0)
        epst = const.tile([128, 1], F32, tag="eps")
        nc.gpsimd.memset(epst[:], EPS)

        # z scratch interleaved [HW, GB, CIN]: full-rate 1KB-row writes; the
        # y re-read pays the small-chunk penalty instead (off critical path)
        zscr = [dram.tile([HW, GB, CIN], BF16, tag=f"zg{g}", name=f"zscr{g}")
                for g in range(NGRP)]
        st_in = dram.tile([128, 4], F32, tag="stin")
        st_gat = dram.tile([NCORES, 128, 4], F32, tag="stgat")

        res_tiles = [[None] * 2 for _ in range(BPC)]

        def read_y(y_sb, g, b4):
            """Reinterpret-read y from interleaved zscr[g] into even/odd
            channel-permuted partition order (pw rows are host-permuted to
            match, so the matmul contraction is unchanged).

            y[2q, 128j+cc] = Z[49q+j, cc]; y[2q+1, ...] starts mid-row 49q+24.
            """
            zv = zscr[g].rearrange("(q s) b c -> q s b c", q=64)
            nc.sync.dma_start(
                y_sb[0:64, 0:24 * CIN].rearrange("p (j c) -> p j c", c=CIN),
                zv[:, 0:24, b4, :])
            nc.sync.dma_start(y_sb[0:64, 24 * CIN:24 * CIN + 64],
                              zv[:, 24, b4, 0:64])
            nc.sync.dma_start(y_sb[64:128, 0:64], zv[:, 24, b4, 64:128])
            nc.sync.dma_start(
                y_sb[64:128, 64:HW].rearrange("p (j c) -> p j c", c=CIN),
                zv[:, 25:49, b4, :])

        # ---- conv / pointwise ----
        # sequential PSUM pools: conv 3x[112,1024] (6 banks) closes before
        # pw 2x[128,2048] (8 banks) opens
        y_tiles = [None] * BPC
        POOL_SQ = set()
        ACT_SQ = {15}
        DVE_RELU = {0}

        def conv_group(g):
            def xt_blk(i):
                j, li = divmod(i, SPAN)
                return xs[g][j][:, 512 * li:512 * (li + 1)]

            # 2-block PSUM tiles: one [112,1024] zcopy per pair, ACT/DVE split
            for i2 in range(NBLK // 2):
                ps = ps_c.tile([BLK, 2 * GB * CIN], F32, tag="cv")
                for k in range(2):
                    i = 2 * i2 + k
                    deltas = [d for d in (-1, 0, 1) if 0 <= i + d < NBLK]
                    for idx, d in enumerate(deltas):
                        nc.tensor.matmul(
                            ps[:, 512 * k:512 * (k + 1)], A[d][:, :],
                            xt_blk(i + d),
                            start=(idx == 0), stop=(idx == len(deltas) - 1))
                z_sb = zpool.tile([BLK, 2 * GB * CIN], BF16, tag="z")
                if i2 % 2 == 0:
                    nc.vector.tensor_copy(z_sb[:], ps[:])
                else:
                    nc.scalar.activation(z_sb[:], ps[:],
                                         mybir.ActivationFunctionType.Copy)
                nc.sync.dma_start(
                    zscr[g][2 * BLK * i2:2 * BLK * (i2 + 1), :, :]
                    .rearrange("(k r) b c -> r k (b c)", k=2),
                    z_sb[:].rearrange("r (k v) -> r k v", k=2))

        def pw_batch(g, b4):
                b = GB * g + b4
                y_sb = y_tiles[b]
                for oc in range(2):
                    res = respool.tile([128, HW], BF16, tag="res")
                    res_tiles[b][oc] = res
                    u = 2 * b + oc
                    # chunk groups (0..3) and (4..6): one 4-bank PSUM tile and
                    # ONE relu+accum instr per group
                    for jj, js in enumerate([(0, 1, 2, 3), (4, 5, 6)]):
                        w = NCH * len(js)
                        ps = ps_p.tile([128, 2048], F32, tag="pw")
                        for k, j in enumerate(js):
                            nc.tensor.matmul(
                                ps[:, 512 * k:512 * k + NCH],
                                pw_sb[:, 128 * oc:128 * (oc + 1)],
                                y_sb[:, NCH * j:NCH * (j + 1)],
                                start=True, stop=True)
                        slot = b * NSL + jj
                        j0 = js[0]
                        ps_in = (ps[:].rearrange("p (k c) -> p k c", c=512)
                                 [:, 0:len(js), 0:NCH])
                        rs = (res[:, NCH * j0:NCH * j0 + w]
                              .rearrange("p (k c) -> p k c", c=NCH))
                        if u in DVE_RELU:
                            nc.vector.tensor_scalar(
                                rs, ps_in, 0.0, 0.0,
                                mybir.AluOpType.max, mybir.AluOpType.add,
                                accum_out=sums[oc][:, slot:slot + 1])
                        else:
                            nc.scalar.activation(
                                rs, ps_in,
                                mybir.ActivationFunctionType.Relu,
                                accum_out=sums[oc][:, slot:slot + 1])
                    # one whole-tile square per unit; sumsq into slot b*NSL
                    sqslot = sqs[oc][:, b * NSL:b * NSL + 1]
                    if u in ACT_SQ:
                        sca = sbpool.tile([128, HW], BF16, tag="sb",
                                          name=f"sa{u}")
                        nc.scalar.activation(
                            sca[:], res[:],
                            mybir.ActivationFunctionType.Square,
                            accum_out=sqslot)
                    elif u in POOL_SQ:
                        scr_big = sbpool.tile([128, HW], BF16, tag="sb",
                                              name=f"sb{u}")
                        nc.gpsimd.tensor_tensor(
                            scr_big[:], res[:], res[:], mybir.AluOpType.mult)
                        sc2 = sbpool.tile([128, HW], BF16, tag="sb",
                                          name=f"sb2_{u}")
                        nc.vector.tensor_scalar(
                            sc2[:], scr_big[:], 1.0, 0.0,
                            mybir.AluOpType.mult, mybir.AluOpType.add,
                            accum_out=sqslot)
                    else:
                        sc = sbpool.tile([128, HW], BF16, tag="sb",
                                         name=f"sc{u}")
                        nc.vector.scalar_tensor_tensor(
                            out=sc[:], in0=res[:], scalar=1.0, in1=res[:],
                            op0=mybir.AluOpType.mult,
                            op1=mybir.AluOpType.mult,
                            accum_out=sqslot)

        # driver: conv(g0) -> y-g0/x-g1 -> conv(g1) -> pw(g0) -> pw(g1)
        ps_conv_ctx = tc.tile_pool(name="ps_c", bufs=3, space="PSUM")
        ps_c = ps_conv_ctx.__enter__()
        conv_group(0)
        for b4 in range(GB):
            y_sb = ypool.tile([128, HW], BF16, tag="y", name=f"y0_{b4}")
            y_tiles[b4] = y_sb
        read_y(y_tiles[0], 0, 0)
        for j in range(NSPAN):
            xt_ = xtpool.tile([BLK, SPAN * GB * CIN], BF16, tag="x",
                              name=f"xs1_{j}")
            nc.sync.dma_start(
                xt_[:].rearrange("p (i c) -> p i c", i=SPAN),
                x_in[1, SPAN * j:SPAN * (j + 1)].rearrange("i p c -> p i c"))
            xs[1][j] = xt_
        for b4 in range(1, GB):
            read_y(y_tiles[b4], 0, b4)
        conv_group(1)
        ps_conv_ctx.__exit__(None, None, None)
        ps_pw_ctx = tc.tile_pool(name="ps_p", bufs=2, space="PSUM")
        ps_p = ps_pw_ctx.__enter__()
        for b4 in range(GB):
            pw_batch(0, b4)
        for b4 in range(GB):
            y_sb = ypool.tile([128, HW], BF16, tag="y", name=f"y1_{b4}")
            y_tiles[GB + b4] = y_sb
            read_y(y_sb, 1, b4)
            pw_batch(1, b4)
        ps_pw_ctx.__exit__(None, None, None)

        # preload the Sqrt act-function table set during the ACT drain so the
        # post-collective chain doesn't pay the table swap
        warm = const.tile([128, 1], F32, tag="warm")
        nc.scalar.activation(warm[:], epst[:],
                             mybir.ActivationFunctionType.Sqrt)

        # ---- stats: local reduce -> AllGather -> local sum -> affine params
        red = const.tile([128, 4], F32, tag="red")
        allr = const.tile([128, 4 * NCORES], F32, tag="allr")
        acc = const.tile([128, 4], F32, tag="acc")
        me = const.tile([128, 4], F32, tag="me")    # mean0 mean1 msq0 msq1
        me2 = const.tile([128, 2], F32, tag="me2")
        var = const.tile([128, 2], F32, tag="var")
        rstd = const.tile([128, 2], F32, tag="rstd")
        sc_b = const.tile([128, 4], F32, tag="scb")  # scale0 scale1 nbias0 nbias1
        for oc in range(2):
            nc.vector.tensor_reduce(red[:, oc:oc + 1], sums[oc][:],
                                    axis=mybir.AxisListType.X,
                                    op=mybir.AluOpType.add)
            nc.vector.tensor_reduce(red[:, 2 + oc:3 + oc], sqs[oc][:],
                                    axis=mybir.AxisListType.X,
                                    op=mybir.AluOpType.add)
        no_cc = bool(os.environ.get("BASS_NO_CC"))
        nc.sync.dma_start(st_in[:], red[:])
        if no_cc:
            for r in range(NCORES):
                nc.sync.dma_start(st_gat[r], st_in[:])
        else:
            nc.gpsimd.collective_compute(
                "AllGather", mybir.AluOpType.bypass,
                replica_groups=[list(range(NCORES))],
                ins=[st_in[:].opt()], outs=[st_gat[:].opt()])
        # r-major readback (16B contiguous per core), then X-reduce over a
        # c-major view so one instruction sums the 8 cores
        nc.sync.dma_start(
            allr[:].rearrange("p (r c) -> p r c", r=NCORES),
            st_gat[:].rearrange("r p c -> p r c"))
        nc.vector.tensor_reduce(
            acc[:], allr[:].rearrange("p (r c) -> p c r", r=NCORES),
            axis=mybir.AxisListType.X, op=mybir.AluOpType.add)
        nc.vector.tensor_scalar(me[:], acc[:], 1.0 / NTOT, None,
                                mybir.AluOpType.mult)
        nc.vector.scalar_tensor_tensor(
            out=me2[:], in0=me[:, 0:2], scalar=1.0, in1=me[:, 0:2],
            op0=mybir.AluOpType.mult, op1=mybir.AluOpType.mult)
        nc.vector.tensor_tensor(var[:], me[:, 2:4], me2[:],
                                mybir.AluOpType.subtract)
        std = const.tile([128, 2], F32, tag="std")
        nc.scalar.activation(std[:], var[:],
                             mybir.ActivationFunctionType.Sqrt,
                             bias=epst[:])
        nc.vector.reciprocal(rstd[:], std[:])
        nc.vector.tensor_tensor(sc_b[:, 0:2], rstd[:], gb_sb[:, 0:2],
                                mybir.AluOpType.mult)
        nc.vector.tensor_tensor(sc_b[:, 2:4], me[:, 0:2], sc_b[:, 0:2],
                                mybir.AluOpType.mult)
        nc.vector.tensor_tensor(sc_b[:, 2:4], gb_sb[:, 2:4], sc_b[:, 2:4],
                                mybir.AluOpType.subtract)

        # ---- phase 2: affine + writeout, DVE/ACT/Pool rotation feeds DMA ----
        stage = int(os.environ.get("BASS_STAGE", "3"))
        for b in range(BPC):
            for oc in range(2):
                o_sb = opool.tile([128, HW], F32, tag="o")
                sel = (2 * b + oc) % 2
                if stage == 2:
                    nc.vector.tensor_copy(o_sb[:], res_tiles[b][oc][:])
                elif sel == 0:
                    nc.vector.tensor_scalar(
                        o_sb[:], res_tiles[b][oc][:],
                        sc_b[:, oc:oc + 1], sc_b[:, 2 + oc:3 + oc],
                        mybir.AluOpType.mult, mybir.AluOpType.add)
                elif sel == 1:
                    nc.scalar.activation(
                        o_sb[:], res_tiles[b][oc][:],
                        mybir.ActivationFunctionType.Identity,
                        bias=sc_b[:, 2 + oc:3 + oc],
                        scale=sc_b[:, oc:oc + 1])
                else:
                    nc.gpsimd.tensor_scalar(
                        o_sb[:], res_tiles[b][oc][:],
                        sc_b[:, oc:oc + 1], sc_b[:, 2 + oc:3 + oc],
                        mybir.AluOpType.mult, mybir.AluOpType.add)
                nc.sync.dma_start(out[b, 128 * oc:128 * (oc + 1), :], o_sb[:])

    nc.finalize()
    return nc


def _build_a3(dw_w):
    """Banded depthwise matrices A_d [rin(p), rout(f)] for d in (-1, 0, 1).

    A_d[p, f] = w[dh,dw] iff p == f + (56*dh+dw) - 112*d and the w-border
    is valid (0 <= f%56 + dw < 56). h-borders are exact via block skipping
    at i=0 / i=27 (cross-row taps there fall outside [0,112) or the w-mask).
    """
    w9 = np.asarray(dw_w, dtype=np.float32).reshape(3, 3)
    a3 = np.zeros((3, BLK, BLK), dtype=np.float32)
    f = np.arange(BLK)
    for di, d in enumerate((-1, 0, 1)):
        for dh in (-1, 0, 1):
            for dw in (-1, 0, 1):
                p = f + (56 * dh + dw) - 112 * d
                valid = (p >= 0) & (p < BLK) & (f % 56 + dw >= 0) & (f % 56 + dw < 56)
                a3[di, p[valid], f[valid]] = w9[dh + 1, dw + 1]
    return a3


_NC_CACHE = []


def kernel(x, dw_w, pw_w, gamma, beta):
    import ml_dtypes
    bf16 = ml_dtypes.bfloat16

    # [64,128,56,56] f32 -> [8 cores, NGRP, 3136, GB, 128] bf16 spatial-major
    xg = (np.asarray(x, dtype=np.float32).astype(bf16)
          .reshape(NCORES, NGRP, GB, CIN, HW)
          .transpose(0, 1, 4, 2, 3))
    xg = np.ascontiguousarray(xg).reshape(NCORES, NGRP, NBLK, BLK, GB * CIN)

    a3 = np.ascontiguousarray(_build_a3(dw_w).astype(bf16))
    # pw rows permuted even-channels-first to match the y-read partition order
    perm = np.r_[0:CIN:2, 1:CIN:2]
    pwT = np.ascontiguousarray(
        np.asarray(pw_w, dtype=np.float32).T.astype(bf16)[perm])
    gamma = np.ascontiguousarray(np.asarray(gamma, dtype=np.float32))
    beta = np.ascontiguousarray(np.asarray(beta, dtype=np.float32))

    if not _NC_CACHE:
        _NC_CACHE.append(build_nc())
    nc = _NC_CACHE[0]

    in_maps = []
    for r in range(NCORES):
        in_maps.append({"xg": np.ascontiguousarray(xg[r]), "a3": a3,
                        "pwT": pwT, "gamma": gamma, "beta": beta})

    br = run_bass_kernel_spmd(nc, in_maps, list(range(NCORES)))
    outs = [br.results[r]["out"].reshape(BPC, COUT, H, W) for r in range(NCORES)]
    return np.concatenate(outs, axis=0)
